# revision 37
# baseline (speedup 1.0000x reference)
"""Causal self-attention Trainium2 kernel (8 NeuronCores).

Sharding: core = (batch b in {0,1}, head-group hg in {0..3}); each core owns
4 of the 16 heads (256 of the 1024 q/k/v dims) for one batch element.
Data parallel over batch, tensor parallel over heads; W_o is row-parallel so
each core emits a partial output that the host sums (+ b_o) at gather time.

Device dataflow (per core), everything in "transposed" layout so the
contraction dim always sits on SBUF partitions:
  xT [1024,2048] bf16, weights pre-transposed+cast on host.
  QT/KT [d=256, s=2048] bf16 (d on partitions, 2 blocks of 128)
  V natural [s, d] bf16 with a ones-column appended per head so the A@V
  matmul also produces the softmax denominator (row 64 of the PSUM tile).

Key perf structure vs the naive version:
  - Attention processes a HEAD PAIR (po=0/po=1 of one dc block) per q tile,
    one k chunk at a time: the two K=64 ST matmuls sit at partition bases
    0/64, land on disjoint PE row groups and run CONCURRENTLY, writing one
    2-bank PSUM pair tile [128,1024]; ONE ACT Exp instruction covers both
    heads (amortizes the ~352-cycle ACTIVATE fixed cost — the exp stream is
    the attention-phase pacer, and the low PE work per exp keeps the
    kernel insensitive to the HAM PE-clock state).
  - Causal masking happens AFTER the exp (exp can't overflow: scores are
    bounded after the 1/8 scale): diagonal chunks are computed only on
    their valid trapezoid [128j:512] q-slice, and a single gpsimd
    affine_select zeroes the remaining upper triangle of both heads.
  - All projection work (Q/K groups per q-tile, V per s-chunk, W_o per
    128-row column, the split last W_o tile) is pumped through a background
    queue, one small group per attention chunk, so the strict-FIFO PE queue
    interleaves it into the ACT-paced gaps; per-block pump plans avoid the
    block boundaries and the norm-dependent first chunks.
  - Normalization: evacuate the av accumulator fast (den row + unnormalized
    out, ~1.4us) so the next block's A@V can start, then a DMA-reshaped
    [128,4] DVE reciprocal + gpsimd partition_broadcast + DVE multiply
    normalize in SBUF off the critical path.
  - QK bias+scale on DVE tensor_scalar (ACT reserved for exps); W_o
    evacuation as bf16 (host sums partials in fp32); xt streamed s-half
    first across two DMA queues so the first half of the schedule is gated
    by 2MB.
"""

import sys

for _p in ("/opt/trn_rl_repo",):
    if _p not in sys.path:
        sys.path.insert(0, _p)

import numpy as np
import ml_dtypes

import concourse.bass as bass
import concourse.bacc as bacc
import concourse.mybir as mybir
from concourse import tile
from concourse.bass_utils import run_bass_kernel_spmd

P = 128
S = 2048  # sequence length
D = 1024  # d_model
DG = 256  # dims per head-group (4 heads x 64)
DH = 64   # head dim
NHG = 4   # heads per core
EC = D // P   # 8 contraction chunks over d_model
KC = S // P   # 16 key chunks
QTW = 512     # q tile width
NQT = S // QTW  # 4 q tiles
W2 = 2 * QTW  # pair-tile width (2 PSUM banks)
F32 = mybir.dt.float32
BF16 = mybir.dt.bfloat16
AF = mybir.ActivationFunctionType

LAST_EXEC_NS = None
LAST_RESULTS = None


def _emit(tc, aps):
    nc = tc.nc
    xt_d, wqt_d, wkt_d, wvt_d, wot_d, bq_d, bk_d, bv_d, yt_d = aps

    with (
        tc.tile_pool(name="const", bufs=1) as constp,
        tc.tile_pool(name="wpool", bufs=1) as wp,
        tc.tile_pool(name="xpool", bufs=1) as xp,
        tc.tile_pool(name="qkvp", bufs=1) as qkvp,
        tc.tile_pool(name="aep", bufs=4) as aep,
        tc.tile_pool(name="outp", bufs=1) as outp,
        tc.tile_pool(name="normp", bufs=2) as normp,
        tc.tile_pool(name="sgp", bufs=3) as sgp,
        tc.tile_pool(name="pspair", bufs=2, space="PSUM") as pspair,
        tc.tile_pool(name="psav", bufs=2, space="PSUM") as psav,
        tc.tile_pool(name="psmisc", bufs=2, space="PSUM") as psmisc,
    ):
        # ---- persistent SBUF tensors ----
        bq_sb = constp.tile([P, 2], F32, name="bq_sb")
        bqs_sb = constp.tile([P, 2], F32, name="bqs_sb")
        bk_sb = constp.tile([P, 2], F32, name="bk_sb")
        bv1_sb = constp.tile([1, DG], F32, name="bv1_sb")
        bvb_sb = constp.tile([P, DG], F32, name="bvb_sb")

        wqt_sb = wp.tile([P, EC, DG], BF16, name="wqt_sb")
        wkt_sb = wp.tile([P, EC, DG], BF16, name="wkt_sb")
        wvt_sb = wp.tile([P, EC, DG], BF16, name="wvt_sb")
        wot_sb = wp.tile([P, 2, D], BF16, name="wot_sb")

        xt_sb = xp.tile([P, EC, S], BF16, name="xt_sb")

        qt_sb = qkvp.tile([P, 2, S], BF16, name="qt_sb")
        kt_sb = qkvp.tile([P, 2, S], BF16, name="kt_sb")
        v_sb = qkvp.tile([P, KC, NHG, DH + 1], BF16, name="v_sb")

        outt_sb = outp.tile([P, 2, S], BF16, name="outt_sb")

        # ---- input DMAs ----
        # xt goes FIRST on the sync queue (in s-halves for finer-grained
        # streaming into the Q/K ec-loops); constants and all weight chunks
        # issue in parallel from the gpsimd queue, so the xt stream is gated
        # by nothing.
        # first s-halves (q tiles 0-1, k chunks 0-7) of all chunks first —
        # the entire first half of the schedule is gated by only these 2MB —
        # alternating between the sync and scalar issue queues so two DGE
        # paths stream the gate in parallel; second halves follow the same
        # way.
        for sh in range(2):
            for ec in range(EC):
                eng = nc.sync if ec % 2 == 0 else nc.scalar
                eng.dma_start(
                    xt_sb[:, ec, sh * S // 2 : (sh + 1) * S // 2],
                    xt_d[ec * P : (ec + 1) * P, sh * S // 2 : (sh + 1) * S // 2],
                )
        for c in range(2):
            nc.gpsimd.dma_start(bq_sb[:, c : c + 1], bq_d[c * P : (c + 1) * P, :])
            nc.gpsimd.dma_start(bk_sb[:, c : c + 1], bk_d[c * P : (c + 1) * P, :])
        nc.gpsimd.dma_start(bv1_sb[:, :], bv_d[:, :])
        nc.scalar.mul(bqs_sb[:, :], bq_sb[:, :], 0.125)
        nc.gpsimd.partition_broadcast(bvb_sb[:, :], bv1_sb[:, :], channels=P)
        # ones column in V for the fused softmax denominator
        nc.vector.memset(v_sb[:, :, :, DH : DH + 1], 1.0)
        for ec in range(EC):
            nc.gpsimd.dma_start(wqt_sb[:, ec, :], wqt_d[ec * P : (ec + 1) * P, :])
        for ec in range(EC):
            nc.gpsimd.dma_start(wkt_sb[:, ec, :], wkt_d[ec * P : (ec + 1) * P, :])
        for ec in range(EC):
            nc.gpsimd.dma_start(wvt_sb[:, ec, :], wvt_d[ec * P : (ec + 1) * P, :])
        for dc in range(2):
            nc.gpsimd.dma_start(wot_sb[:, dc, :], wot_d[dc * P : (dc + 1) * P, :])

        # ---- QKV projections (single-bank groups on the shared "misc"
        # ring, so they can be pumped one at a time between attention
        # chunks and the PE queue interleaves them into the ACT-paced
        # gaps) ----
        def qk_group(w_sb, dst_sb, db, tq4, bias_ap, scale):
            ps = psmisc.tile([P, QTW], F32, name="pqk", tag="misc")
            for ec in range(EC):
                nc.tensor.matmul(
                    ps[:, :],
                    w_sb[:, ec, db * P : (db + 1) * P],
                    xt_sb[:, ec, tq4 * QTW : (tq4 + 1) * QTW],
                    start=(ec == 0),
                    stop=(ec == EC - 1),
                )
            # bias+scale on DVE (tensor_scalar with per-partition scalar AP)
            # so ACT stays reserved for the exp stream.
            nc.vector.tensor_scalar(
                dst_sb[:, db, tq4 * QTW : (tq4 + 1) * QTW],
                ps[:, :],
                scale,
                bias_ap,
                mybir.AluOpType.mult,
                mybir.AluOpType.add,
            )

        # V natural for one s-chunk; DVE adds the (partition-broadcast)
        # bias while casting to bf16 into v_sb.
        def v_group(sc):
            pv = psmisc.tile([P, QTW], F32, name="pv", tag="misc")
            for ec in range(EC):
                nc.tensor.matmul(
                    pv[:, 0:DG],
                    xt_sb[:, ec, sc * P : (sc + 1) * P],
                    wvt_sb[:, ec, :],
                    start=(ec == 0),
                    stop=(ec == EC - 1),
                )
            nc.vector.tensor_add(
                v_sb[:, sc, :, 0:DH],
                pv[:, 0:DG].rearrange("p (h d) -> p h d", h=NHG),
                bvb_sb[:, :].rearrange("p (h d) -> p h d", h=NHG),
            )

        # ---- attention: a head PAIR (po=0, po=1 of one dc block) on one q
        # tile, one k-chunk at a time. The two ST matmuls have K=64 and
        # partition bases 0 / 64, so they land on disjoint PE row groups
        # (tile_position (0,0) / (64,0)) and run CONCURRENTLY; both write one
        # [128,1024] PSUM pair tile -> one Exp ACTIVATE covers both heads ->
        # (diagonal chunks) one gpsimd affine_select zeroes the future
        # entries of both halves -> two A@V matmuls into the two per-head
        # accumulators. PE work per exp is ~3 matmul slots, low enough that
        # even at the cold (K=4/8) PE clock the ACT exp stream stays the
        # pacer, so HAM state stops mattering in this phase.
        def attn_block(t, dc, plan=None):
            hA, hB = 2 * dc, 2 * dc + 1
            cmax = 4 * t + 4
            avs = {
                h: psav.tile([P, QTW], F32, name=f"av{h}", tag="av")
                for h in (hA, hB)
            }

            # Diagonal chunks (c = 4t+j) only need q >= 128j: the ST/exp/
            # affine/AV all run on the trapezoid [qlo:512] slice, qlo = 128j.
            def qlo_of(c):
                return 128 * (c - 4 * t) if c >= 4 * t else 0

            def emit_st(c):
                stp = pspair.tile([P, W2], F32, name="stp", tag="pp")
                qlo = qlo_of(c)
                for i, h in enumerate((hA, hB)):
                    qoff = (h % 2) * DH
                    nc.tensor.matmul(
                        stp[:, i * QTW + qlo : (i + 1) * QTW],
                        kt_sb[qoff : qoff + DH, dc, c * P : (c + 1) * P],
                        qt_sb[qoff : qoff + DH, dc, t * QTW + qlo : (t + 1) * QTW],
                        start=True,
                        stop=True,
                    )
                return stp

            sts = {0: emit_st(0)}
            if cmax > 1:
                sts[1] = emit_st(1)
            for c in range(cmax):
                qlo = qlo_of(c)
                ae = aep.tile([P, W2], BF16, name="ae", tag="ae")
                ae3 = ae[:, :].rearrange("k (h q) -> k h q", h=2)[:, :, qlo:QTW]
                st3 = sts[c][:, :].rearrange("k (h q) -> k h q", h=2)[:, :, qlo:QTW]
                nc.scalar.activation(ae3, st3, AF.Exp)
                if c >= 4 * t:
                    # diagonal chunk: in trapezoid coords keep iff q' >= k
                    nc.gpsimd.affine_select(
                        out=ae3,
                        in_=ae3,
                        compare_op=mybir.AluOpType.is_ge,
                        fill=0.0,
                        base=0,
                        pattern=[[0, 2], [1, QTW - qlo]],
                        channel_multiplier=-1,
                    )
                if c + 2 < cmax:
                    sts[c + 2] = emit_st(c + 2)
                if plan is not None and c < len(plan):
                    for _ in range(plan[c]):
                        pump()
                for i, h in enumerate((hA, hB)):
                    nc.tensor.matmul(
                        avs[h][0 : DH + 1, qlo:QTW],
                        v_sb[:, c, h, :],
                        ae[:, i * QTW + qlo : (i + 1) * QTW],
                        start=(c == 0),
                        stop=(c == cmax - 1),
                    )
            # po=1 head first: its norm ends in a DMA; the po=0 chain ends in
            # a direct DVE multiply, keeping the block tail short.
            last = t == NQT - 1 and dc == 1
            norm_dispatch(hB, t, avs[hB], last)
            norm_dispatch(hA, t, avs[hA], last)

        def norm_dispatch(h, t, av, last=False):
            # av rows 0-63 = unnormalized out, row 64 = denominator.
            # Evacuate the PSUM accumulator FAST (den row + unnormalized out),
            # so the av bank frees after ~1.4us and the next block's A@V can
            # start; the reciprocal chain (DMA reshape [1,512]<->[128,4] so
            # the iterative-divide runs on 128 lanes) then normalizes in SBUF
            # off the critical path. For the LAST block there is no next
            # consumer of the av banks, so skip the staging copy (one less
            # DVE hop on the kernel's tail) and multiply out of PSUM; its
            # small DMAs ride the scalar queue (idle by then).
            dc, po = divmod(h, 2)
            dmae = nc.scalar if last else nc.sync
            tq = slice(t * QTW, (t + 1) * QTW)
            den = normp.tile([1, QTW], F32, name="den", tag="den")
            nc.vector.tensor_copy(den[:, :], av[DH : DH + 1, :])
            if last:
                dst = None
            elif po == 0:
                dst = outt_sb[0:DH, dc, tq]
                nc.vector.tensor_copy(dst, av[0:DH, :])
            else:
                odd = normp.tile([DH, QTW], BF16, name="odd", tag="odd")
                dst = odd[:, :]
                nc.vector.tensor_copy(dst, av[0:DH, :])
            denP = normp.tile([P, 4], F32, name="denP", tag="denP")
            dmae.dma_start(denP[:, :], den[:, :])
            recP = normp.tile([P, 4], F32, name="recP", tag="recP")
            nc.vector.reciprocal(recP[:, :], denP[:, :])
            rec = normp.tile([1, QTW], F32, name="rec", tag="rec")
            dmae.dma_start(rec[:, :], recP[:, :])
            bc = normp.tile([DH, QTW], F32, name="bc", tag="bc")
            nc.gpsimd.partition_broadcast(bc[:, :], rec[:, :], channels=DH)
            if last:
                if po == 0:
                    nc.vector.tensor_mul(outt_sb[0:DH, dc, tq], av[0:DH, :], bc[:, :])
                else:
                    odd = normp.tile([DH, QTW], BF16, name="odd", tag="odd")
                    nc.vector.tensor_mul(odd[:, :], av[0:DH, :], bc[:, :])
                    dmae.dma_start(outt_sb[DH:P, dc, tq], odd[:, :])
            else:
                nc.vector.tensor_mul(dst, dst, bc[:, :])
                if po == 1:
                    nc.sync.dma_start(outt_sb[DH:P, dc, tq], dst)

        # yT[mc, q-tile st4] = sum_dc WoT_chunk.T @ outT_chunk; DVE
        # evacuates PSUM->SBUF as bf16 (halves output DMA bytes; the host
        # gather sums the 4 partials in fp32), then DMA to DRAM. One mc
        # column at a time so the work pumps between attention chunks.
        def wo_mc(st4, mc):
            py = psmisc.tile([P, QTW], F32, name="py", tag="misc")
            for dcw in range(2):
                nc.tensor.matmul(
                    py[:, :],
                    wot_sb[:, dcw, mc * P : (mc + 1) * P],
                    outt_sb[:, dcw, st4 * QTW : (st4 + 1) * QTW],
                    start=(dcw == 0),
                    stop=(dcw == 1),
                )
            sg = sgp.tile([P, QTW], BF16, name="sg", tag="sg")
            nc.vector.tensor_copy(sg[:, :], py[:, :])
            nc.sync.dma_start(
                yt_d[mc * P : (mc + 1) * P, st4 * QTW : (st4 + 1) * QTW],
                sg[:, :],
            )

        # Split form for the LAST tile: the dcw=0 half contraction (heads of
        # dc block 0) pumps into the final attention block; only 8 matmuls +
        # adds + DMAs remain after the final norm.
        wo3_halves = {}

        def wo_mc_half0(st4, mc):
            py = psmisc.tile([P, QTW], F32, name="py", tag="misc")
            nc.tensor.matmul(
                py[:, :],
                wot_sb[:, 0, mc * P : (mc + 1) * P],
                outt_sb[:, 0, st4 * QTW : (st4 + 1) * QTW],
                start=True,
                stop=True,
            )
            sg0 = sgp.tile([P, QTW], F32, name="sg0", tag=f"sg0_{mc}", bufs=1)
            nc.vector.tensor_copy(sg0[:, :], py[:, :])
            wo3_halves[mc] = sg0

        def wo_mc_half1(st4, mc):
            py = psmisc.tile([P, QTW], F32, name="py", tag="misc")
            nc.tensor.matmul(
                py[:, :],
                wot_sb[:, 1, mc * P : (mc + 1) * P],
                outt_sb[:, 1, st4 * QTW : (st4 + 1) * QTW],
                start=True,
                stop=True,
            )
            sg = sgp.tile([P, QTW], BF16, name="sg", tag="sg")
            nc.vector.tensor_add(sg[:, :], py[:, :], wo3_halves[mc][:, :])
            nc.sync.dma_start(
                yt_d[mc * P : (mc + 1) * P, st4 * QTW : (st4 + 1) * QTW],
                sg[:, :],
            )

        # ---- main schedule ----
        # t=0 attention interleaved with the remaining projections so the
        # exp stream (ACT is the long pole) starts as early as possible;
        # attn_block(t, dc) needs only the db=dc Q/K block and V chunks
        # 0..4t+3. wo_tile(t) is emitted one block late so the PE has pair
        # work in flight while tile t's norm chains drain.
        # ---- main schedule ----
        # Just enough projection work up front for the first attention block
        # (Q/K db0 q-tile 0, V chunks 0-1), then a background queue of the
        # remaining projection groups is pumped ONE item per attention chunk,
        # so the PE queue interleaves them into the ACT-paced exp gaps
        # instead of serializing whole blocks. dc=0 blocks (db0-only) run
        # first per tile; W_o tiles one block late; last tile's W_o split
        # around the final attention block.
        def Q(db, tq4):
            b = bqs_sb[:, db : db + 1]
            return lambda: qk_group(wqt_sb, qt_sb, db, tq4, b, 0.125)

        def K(db, tq4):
            b = bk_sb[:, db : db + 1]
            return lambda: qk_group(wkt_sb, kt_sb, db, tq4, b, 1.0)

        def V(sc):
            return lambda: v_group(sc)

        def WO(t, mc):
            return lambda: wo_mc(t, mc)

        bgq = [
            V(2), V(3), K(0, 1), Q(0, 1),
            V(4), V(5), V(6), V(7), K(1, 0), Q(1, 0),
            K(0, 2), Q(0, 2), V(8), V(9), V(10), V(11), K(1, 1), Q(1, 1),
            K(0, 3), Q(0, 3), V(12), V(13), V(14), V(15), K(1, 2), Q(1, 2),
            K(1, 3), Q(1, 3),
            *(WO(0, mc) for mc in range(8)),
            *(WO(1, mc) for mc in range(8)),
            *(WO(2, mc) for mc in range(8)),
            *(lambda mc=mc: wo_mc_half0(3, mc) for mc in range(8)),
        ]

        def pump():
            if bgq:
                bgq.pop(0)()

        Q(0, 0)()
        K(0, 0)()
        v_group(0)
        v_group(1)
        # per-chunk pump plans: no pumping on a block's last two chunks (so
        # the next block's STs aren't delayed at the boundary) and none on
        # the first chunks of blocks whose pumped items depend on the
        # previous block's norms (the W_o / half0 stages).
        attn_block(0, 0, plan=[2, 2, 0, 0])
        attn_block(1, 0, plan=[2, 2, 1, 1, 1, 1, 0, 0])
        attn_block(0, 1, plan=[2, 2, 0, 0])
        attn_block(2, 0, plan=[2, 2, 1, 1, 1, 1, 1, 1, 1, 1, 0, 0])
        attn_block(1, 1, plan=[2, 2, 1, 1, 1, 1, 0, 0])          # wo tile 0
        attn_block(2, 1, plan=[0, 0, 2, 2, 1, 1, 1, 1, 0, 0, 0, 0])  # wo 1
        attn_block(3, 0, plan=[0, 0, 1, 1, 1, 1, 1, 1, 1, 1, 0, 0, 0, 0, 0, 0])  # wo 2
        attn_block(3, 1, plan=[0, 0, 1, 1, 1, 1, 1, 1, 1, 1, 0, 0, 0, 0, 0, 0])  # wo3 half0
        for mc in range(8):
            wo_mc_half1(3, mc)
        assert not bgq, f"{len(bgq)} background items left unpumped"


_NC_CACHE = None


def build_nc():
    global _NC_CACHE
    if _NC_CACHE is not None:
        return _NC_CACHE
    nc = bacc.Bacc("TRN2")
    xt = nc.dram_tensor("xt", [D, S], BF16, kind="ExternalInput")
    wqt = nc.dram_tensor("wqt", [D, DG], BF16, kind="ExternalInput")
    wkt = nc.dram_tensor("wkt", [D, DG], BF16, kind="ExternalInput")
    wvt = nc.dram_tensor("wvt", [D, DG], BF16, kind="ExternalInput")
    wot = nc.dram_tensor("wot", [DG, D], BF16, kind="ExternalInput")
    bq = nc.dram_tensor("bq", [DG, 1], F32, kind="ExternalInput")
    bk = nc.dram_tensor("bk", [DG, 1], F32, kind="ExternalInput")
    bv = nc.dram_tensor("bv", [1, DG], F32, kind="ExternalInput")
    yt = nc.dram_tensor("yt", [D, S], BF16, kind="ExternalOutput")
    aps = tuple(h.ap() for h in (xt, wqt, wkt, wvt, wot, bq, bk, bv, yt))
    with tile.TileContext(nc) as tc:
        _emit(tc, aps)
    nc.finalize()
    _NC_CACHE = nc
    return nc


def make_in_maps(x, W_q, b_q, W_k, b_k, W_v, b_v, W_o):
    bf = ml_dtypes.bfloat16
    in_maps = []
    for core in range(8):
        b, hg = divmod(core, 4)
        sl = slice(hg * DG, (hg + 1) * DG)
        in_maps.append(
            {
                "xt": np.ascontiguousarray(np.asarray(x)[b].T.astype(bf)),
                "wqt": np.ascontiguousarray(np.asarray(W_q)[sl, :].T.astype(bf)),
                "wkt": np.ascontiguousarray(np.asarray(W_k)[sl, :].T.astype(bf)),
                "wvt": np.ascontiguousarray(np.asarray(W_v)[sl, :].T.astype(bf)),
                "wot": np.ascontiguousarray(np.asarray(W_o)[:, sl].T.astype(bf)),
                "bq": np.ascontiguousarray(
                    np.asarray(b_q)[sl].reshape(DG, 1), dtype=np.float32
                ),
                "bk": np.ascontiguousarray(
                    np.asarray(b_k)[sl].reshape(DG, 1), dtype=np.float32
                ),
                "bv": np.ascontiguousarray(
                    np.asarray(b_v)[sl].reshape(1, DG), dtype=np.float32
                ),
            }
        )
    return in_maps


def kernel(x, W_q, b_q, W_k, b_k, W_v, b_v, W_o, b_o, _trace=False):
    global LAST_EXEC_NS, LAST_RESULTS
    nc = build_nc()
    in_maps = make_in_maps(x, W_q, b_q, W_k, b_k, W_v, b_v, W_o)
    kw = {"trace": True} if _trace else {}
    res = run_bass_kernel_spmd(nc, in_maps, core_ids=list(range(8)), **kw)
    LAST_EXEC_NS = res.exec_time_ns
    LAST_RESULTS = res
    b_o = np.asarray(b_o, dtype=np.float32)
    out = np.empty((2, S, D), np.float32)
    for b in range(2):
        ysum = sum(
            np.asarray(res.results[4 * b + g]["yt"], dtype=np.float32)
            for g in range(4)
        )
        out[b] = ysum.T + b_o
    return out


# revision 39
# speedup vs baseline: 1.1783x; 1.1783x over previous
"""Causal self-attention Trainium2 kernel (8 NeuronCores).

Sharding: core = (batch b in {0,1}, head-group hg in {0..3}); each core owns
4 of the 16 heads (256 of the 1024 q/k/v dims) for one batch element.
Data parallel over batch, tensor parallel over heads; W_o is row-parallel so
each core emits a partial output that the host sums (+ b_o) at gather time.

Device dataflow (per core), everything in "transposed" layout so the
contraction dim always sits on SBUF partitions:
  xT [1024,2048] bf16, weights pre-transposed+cast on host.
  QT/KT [d=256, s=2048] bf16 (d on partitions, 2 blocks of 128)
  V natural [s, d] bf16 with a ones-column appended per head so the A@V
  matmul also produces the softmax denominator (row 64 of the PSUM tile).

Key perf structure vs the naive version:
  - Attention processes a HEAD PAIR (po=0/po=1 of one dc block) per q tile,
    one k chunk at a time: the two K=64 ST matmuls sit at partition bases
    0/64, land on disjoint PE row groups and run CONCURRENTLY, writing one
    2-bank PSUM pair tile [128,1024]; ONE ACT Exp instruction covers both
    heads (amortizes the ~352-cycle ACTIVATE fixed cost — the exp stream is
    the attention-phase pacer, and the low PE work per exp keeps the
    kernel insensitive to the HAM PE-clock state).
  - Causal masking happens AFTER the exp (exp can't overflow: scores are
    bounded after the 1/8 scale): diagonal chunks are computed only on
    their valid trapezoid [128j:512] q-slice, and a single gpsimd
    affine_select zeroes the remaining upper triangle of both heads.
  - All projection work (Q/K groups per q-tile, V per s-chunk, W_o per
    128-row column, the split last W_o tile) is pumped through a background
    queue, one small group per attention chunk, so the strict-FIFO PE queue
    interleaves it into the ACT-paced gaps; per-block pump plans avoid the
    block boundaries and the norm-dependent first chunks.
  - Normalization: evacuate the av accumulator fast (den row + unnormalized
    out, ~1.4us) so the next block's A@V can start, then a DMA-reshaped
    [128,4] DVE reciprocal + gpsimd partition_broadcast + DVE multiply
    normalize in SBUF off the critical path.
  - QK bias+scale on DVE tensor_scalar (ACT reserved for exps); W_o
    evacuation as bf16 (host sums partials in fp32); xt streamed s-half
    first across two DMA queues so the first half of the schedule is gated
    by 2MB.
"""

import sys

for _p in ("/opt/trn_rl_repo",):
    if _p not in sys.path:
        sys.path.insert(0, _p)

import numpy as np
import ml_dtypes

import concourse.bass as bass
import concourse.bacc as bacc
import concourse.mybir as mybir
from concourse import tile
from concourse.bass_utils import run_bass_kernel_spmd

P = 128
S = 2048  # sequence length
D = 1024  # d_model
DG = 256  # dims per head-group (4 heads x 64)
DH = 64   # head dim
NHG = 4   # heads per core
EC = D // P   # 8 contraction chunks over d_model
KC = S // P   # 16 key chunks
QTW = 512     # q tile width
NQT = S // QTW  # 4 q tiles
W2 = 2 * QTW  # pair-tile width (2 PSUM banks)
F32 = mybir.dt.float32
BF16 = mybir.dt.bfloat16
AF = mybir.ActivationFunctionType

LAST_EXEC_NS = None
LAST_RESULTS = None


def _emit(tc, aps):
    nc = tc.nc
    xt_d, wqt_d, wkt_d, wvt_d, wot_d, bq_d, bk_d, bv_d, yt_d = aps

    with (
        tc.tile_pool(name="const", bufs=1) as constp,
        tc.tile_pool(name="wpool", bufs=1) as wp,
        tc.tile_pool(name="xpool", bufs=1) as xp,
        tc.tile_pool(name="qkvp", bufs=1) as qkvp,
        tc.tile_pool(name="aep", bufs=4) as aep,
        tc.tile_pool(name="outp", bufs=1) as outp,
        tc.tile_pool(name="normp", bufs=2) as normp,
        tc.tile_pool(name="sgp", bufs=3) as sgp,
        tc.tile_pool(name="pspair", bufs=2, space="PSUM") as pspair,
        tc.tile_pool(name="psav", bufs=2, space="PSUM") as psav,
        tc.tile_pool(name="psmisc", bufs=2, space="PSUM") as psmisc,
    ):
        # ---- persistent SBUF tensors ----
        bq_sb = constp.tile([P, 2], F32, name="bq_sb")
        bqs_sb = constp.tile([P, 2], F32, name="bqs_sb")
        bk_sb = constp.tile([P, 2], F32, name="bk_sb")
        bv1_sb = constp.tile([1, DG], F32, name="bv1_sb")
        bvb_sb = constp.tile([P, DG], F32, name="bvb_sb")

        wqt_sb = wp.tile([P, EC, DG], BF16, name="wqt_sb")
        wkt_sb = wp.tile([P, EC, DG], BF16, name="wkt_sb")
        wvt_sb = wp.tile([P, EC, DG], BF16, name="wvt_sb")
        wot_sb = wp.tile([P, 2, D], BF16, name="wot_sb")

        xt_sb = xp.tile([P, EC, S], BF16, name="xt_sb")

        qt_sb = qkvp.tile([P, 2, S], BF16, name="qt_sb")
        kt_sb = qkvp.tile([P, 2, S], BF16, name="kt_sb")
        v_sb = qkvp.tile([P, KC, NHG, DH + 1], BF16, name="v_sb")

        outt_sb = outp.tile([P, 2, S], BF16, name="outt_sb")

        # ---- input DMAs ----
        # xt goes FIRST on the sync queue (in s-halves for finer-grained
        # streaming into the Q/K ec-loops); constants and all weight chunks
        # issue in parallel from the gpsimd queue, so the xt stream is gated
        # by nothing.
        # first s-halves (q tiles 0-1, k chunks 0-7) of all chunks first —
        # the entire first half of the schedule is gated by only these 2MB —
        # alternating between the sync and scalar issue queues so two DGE
        # paths stream the gate in parallel; second halves follow the same
        # way.
        for sh in range(2):
            for ec in range(EC):
                eng = nc.sync if ec % 2 == 0 else nc.scalar
                eng.dma_start(
                    xt_sb[:, ec, sh * S // 2 : (sh + 1) * S // 2],
                    xt_d[ec * P : (ec + 1) * P, sh * S // 2 : (sh + 1) * S // 2],
                )
        for c in range(2):
            nc.gpsimd.dma_start(bq_sb[:, c : c + 1], bq_d[c * P : (c + 1) * P, :])
            nc.gpsimd.dma_start(bk_sb[:, c : c + 1], bk_d[c * P : (c + 1) * P, :])
        nc.gpsimd.dma_start(bv1_sb[:, :], bv_d[:, :])
        nc.scalar.mul(bqs_sb[:, :], bq_sb[:, :], 0.125)
        nc.gpsimd.partition_broadcast(bvb_sb[:, :], bv1_sb[:, :], channels=P)
        # ones column in V for the fused softmax denominator
        nc.vector.memset(v_sb[:, :, :, DH : DH + 1], 1.0)
        for ec in range(EC):
            nc.gpsimd.dma_start(wqt_sb[:, ec, :], wqt_d[ec * P : (ec + 1) * P, :])
        for ec in range(EC):
            nc.gpsimd.dma_start(wkt_sb[:, ec, :], wkt_d[ec * P : (ec + 1) * P, :])
        for ec in range(EC):
            nc.gpsimd.dma_start(wvt_sb[:, ec, :], wvt_d[ec * P : (ec + 1) * P, :])
        for dc in range(2):
            nc.gpsimd.dma_start(wot_sb[:, dc, :], wot_d[dc * P : (dc + 1) * P, :])

        # ---- QKV projections (single-bank groups on the shared "misc"
        # ring, so they can be pumped one at a time between attention
        # chunks and the PE queue interleaves them into the ACT-paced
        # gaps) ----
        def qk_group(w_sb, dst_sb, db, tq4, bias_ap, scale):
            ps = psmisc.tile([P, QTW], F32, name="pqk", tag="misc")
            for ec in range(EC):
                nc.tensor.matmul(
                    ps[:, :],
                    w_sb[:, ec, db * P : (db + 1) * P],
                    xt_sb[:, ec, tq4 * QTW : (tq4 + 1) * QTW],
                    start=(ec == 0),
                    stop=(ec == EC - 1),
                )
            # bias+scale on DVE (tensor_scalar with per-partition scalar AP)
            # so ACT stays reserved for the exp stream.
            nc.vector.tensor_scalar(
                dst_sb[:, db, tq4 * QTW : (tq4 + 1) * QTW],
                ps[:, :],
                scale,
                bias_ap,
                mybir.AluOpType.mult,
                mybir.AluOpType.add,
            )

        # V natural for one s-chunk; DVE adds the (partition-broadcast)
        # bias while casting to bf16 into v_sb.
        def v_group(sc):
            pv = psmisc.tile([P, QTW], F32, name="pv", tag="misc")
            for ec in range(EC):
                nc.tensor.matmul(
                    pv[:, 0:DG],
                    xt_sb[:, ec, sc * P : (sc + 1) * P],
                    wvt_sb[:, ec, :],
                    start=(ec == 0),
                    stop=(ec == EC - 1),
                )
            nc.vector.tensor_add(
                v_sb[:, sc, :, 0:DH],
                pv[:, 0:DG].rearrange("p (h d) -> p h d", h=NHG),
                bvb_sb[:, :].rearrange("p (h d) -> p h d", h=NHG),
            )

        # ---- attention: a head PAIR (po=0, po=1 of one dc block) on one q
        # tile, one k-chunk at a time. The two ST matmuls have K=64 and
        # partition bases 0 / 64, so they land on disjoint PE row groups
        # (tile_position (0,0) / (64,0)) and run CONCURRENTLY; both write one
        # [128,1024] PSUM pair tile -> one Exp ACTIVATE covers both heads ->
        # (diagonal chunks) one gpsimd affine_select zeroes the future
        # entries of both halves -> two A@V matmuls into the two per-head
        # accumulators. PE work per exp is ~3 matmul slots, low enough that
        # even at the cold (K=4/8) PE clock the ACT exp stream stays the
        # pacer, so HAM state stops mattering in this phase.
        def attn_block(t, dc, plan=None):
            hA, hB = 2 * dc, 2 * dc + 1
            cmax = 4 * t + 4
            avs = {
                h: psav.tile([P, QTW], F32, name=f"av{h}", tag="av")
                for h in (hA, hB)
            }

            # Diagonal chunks (c = 4t+j) only need q >= 128j: the ST/exp/
            # affine/AV all run on the trapezoid [qlo:512] slice, qlo = 128j.
            def qlo_of(c):
                return 128 * (c - 4 * t) if c >= 4 * t else 0

            def emit_st(c):
                stp = pspair.tile([P, W2], F32, name="stp", tag="pp")
                qlo = qlo_of(c)
                for i, h in enumerate((hA, hB)):
                    qoff = (h % 2) * DH
                    nc.tensor.matmul(
                        stp[:, i * QTW + qlo : (i + 1) * QTW],
                        kt_sb[qoff : qoff + DH, dc, c * P : (c + 1) * P],
                        qt_sb[qoff : qoff + DH, dc, t * QTW + qlo : (t + 1) * QTW],
                        start=True,
                        stop=True,
                    )
                return stp

            sts = {0: emit_st(0)}
            if cmax > 1:
                sts[1] = emit_st(1)
            for c in range(cmax):
                qlo = qlo_of(c)
                ae = aep.tile([P, W2], BF16, name="ae", tag="ae")
                ae3 = ae[:, :].rearrange("k (h q) -> k h q", h=2)[:, :, qlo:QTW]
                st3 = sts[c][:, :].rearrange("k (h q) -> k h q", h=2)[:, :, qlo:QTW]
                nc.scalar.activation(ae3, st3, AF.Exp)
                if c >= 4 * t:
                    # diagonal chunk: in trapezoid coords keep iff q' >= k
                    nc.gpsimd.affine_select(
                        out=ae3,
                        in_=ae3,
                        compare_op=mybir.AluOpType.is_ge,
                        fill=0.0,
                        base=0,
                        pattern=[[0, 2], [1, QTW - qlo]],
                        channel_multiplier=-1,
                    )
                if c + 2 < cmax:
                    sts[c + 2] = emit_st(c + 2)
                if plan is not None and c < len(plan):
                    for _ in range(plan[c]):
                        pump()
                for i, h in enumerate((hA, hB)):
                    nc.tensor.matmul(
                        avs[h][0 : DH + 1, qlo:QTW],
                        v_sb[:, c, h, :],
                        ae[:, i * QTW + qlo : (i + 1) * QTW],
                        start=(c == 0),
                        stop=(c == cmax - 1),
                    )
            # po=1 head first: its norm ends in a DMA; the po=0 chain ends in
            # a direct DVE multiply, keeping the block tail short.
            last = t == NQT - 1 and dc == 1
            norm_dispatch(hB, t, avs[hB], last)
            norm_dispatch(hA, t, avs[hA], last)

        def norm_dispatch(h, t, av, last=False):
            # av rows 0-63 = unnormalized out, row 64 = denominator.
            # Evacuate the PSUM accumulator FAST (den row + unnormalized out),
            # so the av bank frees after ~1.4us and the next block's A@V can
            # start; the reciprocal chain (DMA reshape [1,512]<->[128,4] so
            # the iterative-divide runs on 128 lanes) then normalizes in SBUF
            # off the critical path. For the LAST block there is no next
            # consumer of the av banks, so skip the staging copy (one less
            # DVE hop on the kernel's tail) and multiply out of PSUM; its
            # small DMAs ride the scalar queue (idle by then).
            dc, po = divmod(h, 2)
            dmae = nc.scalar if last else nc.sync
            tq = slice(t * QTW, (t + 1) * QTW)
            den = normp.tile([1, QTW], F32, name="den", tag="den")
            nc.vector.tensor_copy(den[:, :], av[DH : DH + 1, :])
            if last:
                dst = None
            elif po == 0:
                dst = outt_sb[0:DH, dc, tq]
                nc.vector.tensor_copy(dst, av[0:DH, :])
            else:
                odd = normp.tile([DH, QTW], BF16, name="odd", tag="odd")
                dst = odd[:, :]
                nc.vector.tensor_copy(dst, av[0:DH, :])
            denP = normp.tile([P, 4], F32, name="denP", tag="denP")
            dmae.dma_start(denP[:, :], den[:, :])
            recP = normp.tile([P, 4], F32, name="recP", tag="recP")
            nc.vector.reciprocal(recP[:, :], denP[:, :])
            rec = normp.tile([1, QTW], F32, name="rec", tag="rec")
            dmae.dma_start(rec[:, :], recP[:, :])
            bc = normp.tile([DH, QTW], F32, name="bc", tag="bc")
            nc.gpsimd.partition_broadcast(bc[:, :], rec[:, :], channels=DH)
            if last:
                if po == 0:
                    nc.vector.tensor_mul(outt_sb[0:DH, dc, tq], av[0:DH, :], bc[:, :])
                else:
                    odd = normp.tile([DH, QTW], BF16, name="odd", tag="odd")
                    nc.vector.tensor_mul(odd[:, :], av[0:DH, :], bc[:, :])
                    dmae.dma_start(outt_sb[DH:P, dc, tq], odd[:, :])
            else:
                nc.vector.tensor_mul(dst, dst, bc[:, :])
                if po == 1:
                    nc.sync.dma_start(outt_sb[DH:P, dc, tq], dst)

        # yT[mc, q-tile st4] = sum_dc WoT_chunk.T @ outT_chunk; DVE
        # evacuates PSUM->SBUF as bf16 (halves output DMA bytes; the host
        # gather sums the 4 partials in fp32), then DMA to DRAM. One mc
        # column at a time so the work pumps between attention chunks.
        def wo_mc(st4, mc):
            py = psmisc.tile([P, QTW], F32, name="py", tag="misc")
            for dcw in range(2):
                nc.tensor.matmul(
                    py[:, :],
                    wot_sb[:, dcw, mc * P : (mc + 1) * P],
                    outt_sb[:, dcw, st4 * QTW : (st4 + 1) * QTW],
                    start=(dcw == 0),
                    stop=(dcw == 1),
                )
            sg = sgp.tile([P, QTW], BF16, name="sg", tag="sg")
            nc.vector.tensor_copy(sg[:, :], py[:, :])
            nc.sync.dma_start(
                yt_d[mc * P : (mc + 1) * P, st4 * QTW : (st4 + 1) * QTW],
                sg[:, :],
            )

        # Split form for the LAST tile: the dcw=0 half contraction (heads of
        # dc block 0) pumps into the final attention block; only 8 matmuls +
        # adds + DMAs remain after the final norm.
        wo3_halves = {}

        def wo_mc_half0(st4, mc):
            py = psmisc.tile([P, QTW], F32, name="py", tag="misc")
            nc.tensor.matmul(
                py[:, :],
                wot_sb[:, 0, mc * P : (mc + 1) * P],
                outt_sb[:, 0, st4 * QTW : (st4 + 1) * QTW],
                start=True,
                stop=True,
            )
            sg0 = sgp.tile([P, QTW], F32, name="sg0", tag=f"sg0_{mc}", bufs=1)
            nc.vector.tensor_copy(sg0[:, :], py[:, :])
            wo3_halves[mc] = sg0

        def wo_mc_half1(st4, mc):
            py = psmisc.tile([P, QTW], F32, name="py", tag="misc")
            nc.tensor.matmul(
                py[:, :],
                wot_sb[:, 1, mc * P : (mc + 1) * P],
                outt_sb[:, 1, st4 * QTW : (st4 + 1) * QTW],
                start=True,
                stop=True,
            )
            sg = sgp.tile([P, QTW], BF16, name="sg", tag="sg")
            nc.vector.tensor_add(sg[:, :], py[:, :], wo3_halves[mc][:, :])
            nc.sync.dma_start(
                yt_d[mc * P : (mc + 1) * P, st4 * QTW : (st4 + 1) * QTW],
                sg[:, :],
            )

        # ---- main schedule ----
        # t=0 attention interleaved with the remaining projections so the
        # exp stream (ACT is the long pole) starts as early as possible;
        # attn_block(t, dc) needs only the db=dc Q/K block and V chunks
        # 0..4t+3. wo_tile(t) is emitted one block late so the PE has pair
        # work in flight while tile t's norm chains drain.
        # ---- main schedule ----
        # Just enough projection work up front for the first attention block
        # (Q/K db0 q-tile 0, V chunks 0-1), then a background queue of the
        # remaining projection groups is pumped ONE item per attention chunk,
        # so the PE queue interleaves them into the ACT-paced exp gaps
        # instead of serializing whole blocks. dc=0 blocks (db0-only) run
        # first per tile; W_o tiles one block late; last tile's W_o split
        # around the final attention block.
        def Q(db, tq4):
            b = bqs_sb[:, db : db + 1]
            return lambda: qk_group(wqt_sb, qt_sb, db, tq4, b, 0.125)

        def K(db, tq4):
            b = bk_sb[:, db : db + 1]
            return lambda: qk_group(wkt_sb, kt_sb, db, tq4, b, 1.0)

        def V(sc):
            return lambda: v_group(sc)

        def WO(t, mc):
            return lambda: wo_mc(t, mc)

        bgq = [
            V(2), V(3), K(0, 1), Q(0, 1),
            V(4), V(5), V(6), V(7), K(1, 0), Q(1, 0),
            K(0, 2), Q(0, 2), V(8), V(9),
            K(1, 1), Q(1, 1), K(0, 3), Q(0, 3),
            K(1, 2), Q(1, 2), K(1, 3), Q(1, 3), V(10), V(11),
            *(WO(0, mc) for mc in range(8)),
            *(WO(1, mc) for mc in range(8)),
            V(12), V(13),
            V(14), V(15),
            *(WO(2, mc) for mc in range(8)),
            *(lambda mc=mc: wo_mc_half0(3, mc) for mc in range(8)),
        ]

        def pump():
            if bgq:
                bgq.pop(0)()

        Q(0, 0)()
        K(0, 0)()
        v_group(0)
        v_group(1)
        # per-chunk pump plans: no pumping on a block's last two chunks (so
        # the next block's STs aren't delayed at the boundary) and none on
        # the first chunks of blocks whose pumped items depend on the
        # previous block's norms (the W_o / half0 stages).
        attn_block(0, 0, plan=[2, 2, 0, 0])
        attn_block(1, 0, plan=[2, 2, 1, 1, 1, 1, 0, 0])
        attn_block(0, 1, plan=[1, 1, 0, 0])                    # V8-9
        attn_block(2, 0, plan=[2, 2, 1, 1, 1, 1, 1, 1, 0, 0, 0, 0])  # K/Q rest + V10-11
        attn_block(1, 1, plan=[2, 2, 1, 1, 1, 1, 0, 0])        # wo tile 0
        attn_block(2, 1, plan=[0, 0, 2, 2, 1, 1, 1, 1, 1, 1, 0, 0])  # wo1 + V12-13
        attn_block(3, 0, plan=[1, 1, 1, 1, 1, 1, 1, 1, 1, 1, 0, 0, 0, 0, 0, 0])  # V14-15 + wo2
        attn_block(3, 1, plan=[0, 0, 1, 1, 1, 1, 1, 1, 1, 1, 0, 0, 0, 0, 0, 0])  # wo3 half0
        for mc in range(8):
            wo_mc_half1(3, mc)
        assert not bgq, f"{len(bgq)} background items left unpumped"


_NC_CACHE = None


def build_nc():
    global _NC_CACHE
    if _NC_CACHE is not None:
        return _NC_CACHE
    nc = bacc.Bacc("TRN2")
    xt = nc.dram_tensor("xt", [D, S], BF16, kind="ExternalInput")
    wqt = nc.dram_tensor("wqt", [D, DG], BF16, kind="ExternalInput")
    wkt = nc.dram_tensor("wkt", [D, DG], BF16, kind="ExternalInput")
    wvt = nc.dram_tensor("wvt", [D, DG], BF16, kind="ExternalInput")
    wot = nc.dram_tensor("wot", [DG, D], BF16, kind="ExternalInput")
    bq = nc.dram_tensor("bq", [DG, 1], F32, kind="ExternalInput")
    bk = nc.dram_tensor("bk", [DG, 1], F32, kind="ExternalInput")
    bv = nc.dram_tensor("bv", [1, DG], F32, kind="ExternalInput")
    yt = nc.dram_tensor("yt", [D, S], BF16, kind="ExternalOutput")
    aps = tuple(h.ap() for h in (xt, wqt, wkt, wvt, wot, bq, bk, bv, yt))
    with tile.TileContext(nc) as tc:
        _emit(tc, aps)
    nc.finalize()
    _NC_CACHE = nc
    return nc


def make_in_maps(x, W_q, b_q, W_k, b_k, W_v, b_v, W_o):
    bf = ml_dtypes.bfloat16
    in_maps = []
    for core in range(8):
        b, hg = divmod(core, 4)
        sl = slice(hg * DG, (hg + 1) * DG)
        in_maps.append(
            {
                "xt": np.ascontiguousarray(np.asarray(x)[b].T.astype(bf)),
                "wqt": np.ascontiguousarray(np.asarray(W_q)[sl, :].T.astype(bf)),
                "wkt": np.ascontiguousarray(np.asarray(W_k)[sl, :].T.astype(bf)),
                "wvt": np.ascontiguousarray(np.asarray(W_v)[sl, :].T.astype(bf)),
                "wot": np.ascontiguousarray(np.asarray(W_o)[:, sl].T.astype(bf)),
                "bq": np.ascontiguousarray(
                    np.asarray(b_q)[sl].reshape(DG, 1), dtype=np.float32
                ),
                "bk": np.ascontiguousarray(
                    np.asarray(b_k)[sl].reshape(DG, 1), dtype=np.float32
                ),
                "bv": np.ascontiguousarray(
                    np.asarray(b_v)[sl].reshape(1, DG), dtype=np.float32
                ),
            }
        )
    return in_maps


def kernel(x, W_q, b_q, W_k, b_k, W_v, b_v, W_o, b_o, _trace=False):
    global LAST_EXEC_NS, LAST_RESULTS
    nc = build_nc()
    in_maps = make_in_maps(x, W_q, b_q, W_k, b_k, W_v, b_v, W_o)
    kw = {"trace": True} if _trace else {}
    res = run_bass_kernel_spmd(nc, in_maps, core_ids=list(range(8)), **kw)
    LAST_EXEC_NS = res.exec_time_ns
    LAST_RESULTS = res
    b_o = np.asarray(b_o, dtype=np.float32)
    out = np.empty((2, S, D), np.float32)
    for b in range(2):
        ysum = sum(
            np.asarray(res.results[4 * b + g]["yt"], dtype=np.float32)
            for g in range(4)
        )
        out[b] = ysum.T + b_o
    return out


# revision 41
# speedup vs baseline: 1.1879x; 1.0082x over previous
"""Causal self-attention Trainium2 kernel (8 NeuronCores).

Sharding: core = (batch b in {0,1}, head-group hg in {0..3}); each core owns
4 of the 16 heads (256 of the 1024 q/k/v dims) for one batch element.
Data parallel over batch, tensor parallel over heads; W_o is row-parallel so
each core emits a partial output that the host sums (+ b_o) at gather time.

Device dataflow (per core), everything in "transposed" layout so the
contraction dim always sits on SBUF partitions:
  xT [1024,2048] bf16, weights pre-transposed+cast on host.
  QT/KT [d=256, s=2048] bf16 (d on partitions, 2 blocks of 128)
  V natural [s, d] bf16 with a ones-column appended per head so the A@V
  matmul also produces the softmax denominator (row 64 of the PSUM tile).

Key perf structure vs the naive version:
  - Attention processes a HEAD PAIR (po=0/po=1 of one dc block) per q tile,
    one k chunk at a time: the two K=64 ST matmuls sit at partition bases
    0/64, land on disjoint PE row groups and run CONCURRENTLY, writing one
    2-bank PSUM pair tile [128,1024]; ONE ACT Exp instruction covers both
    heads (amortizes the ~352-cycle ACTIVATE fixed cost — the exp stream is
    the attention-phase pacer, and the low PE work per exp keeps the
    kernel insensitive to the HAM PE-clock state).
  - Causal masking happens AFTER the exp (exp can't overflow: scores are
    bounded after the 1/8 scale): diagonal chunks are computed only on
    their valid trapezoid [128j:512] q-slice, and a single gpsimd
    affine_select zeroes the remaining upper triangle of both heads.
  - All projection work (Q/K groups per q-tile, V per s-chunk, W_o per
    128-row column, the split last W_o tile) is pumped through a background
    queue, one small group per attention chunk, so the strict-FIFO PE queue
    interleaves it into the ACT-paced gaps; per-block pump plans avoid the
    block boundaries and the norm-dependent first chunks.
  - Normalization: evacuate the av accumulator fast (den row + unnormalized
    out, ~1.4us) so the next block's A@V can start, then a DMA-reshaped
    [128,4] DVE reciprocal + gpsimd partition_broadcast + DVE multiply
    normalize in SBUF off the critical path.
  - QK bias+scale on DVE tensor_scalar (ACT reserved for exps); W_o
    evacuation as bf16 (host sums partials in fp32); xt streamed s-half
    first across two DMA queues so the first half of the schedule is gated
    by 2MB.
"""

import sys

for _p in ("/opt/trn_rl_repo",):
    if _p not in sys.path:
        sys.path.insert(0, _p)

import numpy as np
import ml_dtypes

import concourse.bass as bass
import concourse.bacc as bacc
import concourse.mybir as mybir
from concourse import tile
from concourse.bass_utils import run_bass_kernel_spmd

P = 128
S = 2048  # sequence length
D = 1024  # d_model
DG = 256  # dims per head-group (4 heads x 64)
DH = 64   # head dim
NHG = 4   # heads per core
EC = D // P   # 8 contraction chunks over d_model
KC = S // P   # 16 key chunks
QTW = 512     # q tile width
NQT = S // QTW  # 4 q tiles
W2 = 2 * QTW  # pair-tile width (2 PSUM banks)
F32 = mybir.dt.float32
BF16 = mybir.dt.bfloat16
AF = mybir.ActivationFunctionType

LAST_EXEC_NS = None
LAST_RESULTS = None


def _emit(tc, aps):
    nc = tc.nc
    xt_d, wqt_d, wkt_d, wvt_d, wot_d, bq_d, bk_d, bv_d, yt_d = aps

    with (
        tc.tile_pool(name="const", bufs=1) as constp,
        tc.tile_pool(name="wpool", bufs=1) as wp,
        tc.tile_pool(name="xpool", bufs=1) as xp,
        tc.tile_pool(name="qkvp", bufs=1) as qkvp,
        tc.tile_pool(name="aep", bufs=4) as aep,
        tc.tile_pool(name="outp", bufs=1) as outp,
        tc.tile_pool(name="normp", bufs=2) as normp,
        tc.tile_pool(name="sgp", bufs=3) as sgp,
        tc.tile_pool(name="pspair", bufs=2, space="PSUM") as pspair,
        tc.tile_pool(name="psav", bufs=2, space="PSUM") as psav,
        tc.tile_pool(name="psmisc", bufs=2, space="PSUM") as psmisc,
    ):
        # ---- persistent SBUF tensors ----
        bq_sb = constp.tile([P, 2], F32, name="bq_sb")
        bqs_sb = constp.tile([P, 2], F32, name="bqs_sb")
        bk_sb = constp.tile([P, 2], F32, name="bk_sb")
        bv1_sb = constp.tile([1, DG], F32, name="bv1_sb")
        bvb_sb = constp.tile([P, DG], F32, name="bvb_sb")

        wqt_sb = wp.tile([P, EC, DG], BF16, name="wqt_sb")
        wkt_sb = wp.tile([P, EC, DG], BF16, name="wkt_sb")
        wvt_sb = wp.tile([P, EC, DG], BF16, name="wvt_sb")
        wot_sb = wp.tile([P, 2, D], BF16, name="wot_sb")

        xt_sb = xp.tile([P, EC, S], BF16, name="xt_sb")

        qt_sb = qkvp.tile([P, 2, S], BF16, name="qt_sb")
        kt_sb = qkvp.tile([P, 2, S], BF16, name="kt_sb")
        v_sb = qkvp.tile([P, KC, NHG, DH + 1], BF16, name="v_sb")

        outt_sb = outp.tile([P, 2, S], BF16, name="outt_sb")

        # ---- input DMAs ----
        # xt goes FIRST on the sync queue (in s-halves for finer-grained
        # streaming into the Q/K ec-loops); constants and all weight chunks
        # issue in parallel from the gpsimd queue, so the xt stream is gated
        # by nothing.
        # first s-halves (q tiles 0-1, k chunks 0-7) of all chunks first —
        # the entire first half of the schedule is gated by only these 2MB.
        # BATCHED transfers (1MB each, split across the sync and scalar
        # queues): per-chunk DMAs fragment across the DGE round-robin and
        # the last chunks land ~10us late; one descriptor per 4-chunk group
        # streams contiguously.
        for sh in range(2):
            for eg in range(2):
                eng = nc.sync if eg == 0 else nc.scalar
                e0 = eg * (EC // 2)
                eng.dma_start(
                    xt_sb[:, e0 : e0 + EC // 2, sh * S // 2 : (sh + 1) * S // 2],
                    xt_d[e0 * P : (e0 + EC // 2) * P, sh * S // 2 : (sh + 1) * S // 2]
                    .rearrange("(ec p) s -> p ec s", p=P),
                )
        for c in range(2):
            nc.gpsimd.dma_start(bq_sb[:, c : c + 1], bq_d[c * P : (c + 1) * P, :])
            nc.gpsimd.dma_start(bk_sb[:, c : c + 1], bk_d[c * P : (c + 1) * P, :])
        nc.gpsimd.dma_start(bv1_sb[:, :], bv_d[:, :])
        nc.scalar.mul(bqs_sb[:, :], bq_sb[:, :], 0.125)
        nc.gpsimd.partition_broadcast(bvb_sb[:, :], bv1_sb[:, :], channels=P)
        # ones column in V for the fused softmax denominator
        nc.vector.memset(v_sb[:, :, :, DH : DH + 1], 1.0)
        # one batched DMA per weight tensor (fewer descriptors competing
        # with the xt stream)
        nc.gpsimd.dma_start(
            wqt_sb[:, :, :], wqt_d[:, :].rearrange("(ec p) d -> p ec d", p=P)
        )
        nc.gpsimd.dma_start(
            wkt_sb[:, :, :], wkt_d[:, :].rearrange("(ec p) d -> p ec d", p=P)
        )
        nc.gpsimd.dma_start(
            wvt_sb[:, :, :], wvt_d[:, :].rearrange("(ec p) d -> p ec d", p=P)
        )
        nc.gpsimd.dma_start(
            wot_sb[:, :, :], wot_d[:, :].rearrange("(c p) m -> p c m", p=P)
        )

        # ---- QKV projections (single-bank groups on the shared "misc"
        # ring, so they can be pumped one at a time between attention
        # chunks and the PE queue interleaves them into the ACT-paced
        # gaps) ----
        def qk_group(w_sb, dst_sb, db, tq4, bias_ap, scale):
            ps = psmisc.tile([P, QTW], F32, name="pqk", tag="misc")
            for ec in range(EC):
                nc.tensor.matmul(
                    ps[:, :],
                    w_sb[:, ec, db * P : (db + 1) * P],
                    xt_sb[:, ec, tq4 * QTW : (tq4 + 1) * QTW],
                    start=(ec == 0),
                    stop=(ec == EC - 1),
                )
            # bias+scale on DVE (tensor_scalar with per-partition scalar AP)
            # so ACT stays reserved for the exp stream.
            nc.vector.tensor_scalar(
                dst_sb[:, db, tq4 * QTW : (tq4 + 1) * QTW],
                ps[:, :],
                scale,
                bias_ap,
                mybir.AluOpType.mult,
                mybir.AluOpType.add,
            )

        # V natural for one s-chunk; DVE adds the (partition-broadcast)
        # bias while casting to bf16 into v_sb.
        def v_group(sc):
            pv = psmisc.tile([P, QTW], F32, name="pv", tag="misc")
            for ec in range(EC):
                nc.tensor.matmul(
                    pv[:, 0:DG],
                    xt_sb[:, ec, sc * P : (sc + 1) * P],
                    wvt_sb[:, ec, :],
                    start=(ec == 0),
                    stop=(ec == EC - 1),
                )
            nc.vector.tensor_add(
                v_sb[:, sc, :, 0:DH],
                pv[:, 0:DG].rearrange("p (h d) -> p h d", h=NHG),
                bvb_sb[:, :].rearrange("p (h d) -> p h d", h=NHG),
            )

        # ---- attention: a head PAIR (po=0, po=1 of one dc block) on one q
        # tile, one k-chunk at a time. The two ST matmuls have K=64 and
        # partition bases 0 / 64, so they land on disjoint PE row groups
        # (tile_position (0,0) / (64,0)) and run CONCURRENTLY; both write one
        # [128,1024] PSUM pair tile -> one Exp ACTIVATE covers both heads ->
        # (diagonal chunks) one gpsimd affine_select zeroes the future
        # entries of both halves -> two A@V matmuls into the two per-head
        # accumulators. PE work per exp is ~3 matmul slots, low enough that
        # even at the cold (K=4/8) PE clock the ACT exp stream stays the
        # pacer, so HAM state stops mattering in this phase.
        def attn_block(t, dc, plan=None):
            hA, hB = 2 * dc, 2 * dc + 1
            cmax = 4 * t + 4
            avs = {
                h: psav.tile([P, QTW], F32, name=f"av{h}", tag="av")
                for h in (hA, hB)
            }

            # Diagonal chunks (c = 4t+j) only need q >= 128j: the ST/exp/
            # affine/AV all run on the trapezoid [qlo:512] slice, qlo = 128j.
            def qlo_of(c):
                return 128 * (c - 4 * t) if c >= 4 * t else 0

            def emit_st(c):
                stp = pspair.tile([P, W2], F32, name="stp", tag="pp")
                qlo = qlo_of(c)
                for i, h in enumerate((hA, hB)):
                    qoff = (h % 2) * DH
                    nc.tensor.matmul(
                        stp[:, i * QTW + qlo : (i + 1) * QTW],
                        kt_sb[qoff : qoff + DH, dc, c * P : (c + 1) * P],
                        qt_sb[qoff : qoff + DH, dc, t * QTW + qlo : (t + 1) * QTW],
                        start=True,
                        stop=True,
                    )
                return stp

            sts = {0: emit_st(0)}
            if cmax > 1:
                sts[1] = emit_st(1)
            for c in range(cmax):
                qlo = qlo_of(c)
                ae = aep.tile([P, W2], BF16, name="ae", tag="ae")
                ae3 = ae[:, :].rearrange("k (h q) -> k h q", h=2)[:, :, qlo:QTW]
                st3 = sts[c][:, :].rearrange("k (h q) -> k h q", h=2)[:, :, qlo:QTW]
                nc.scalar.activation(ae3, st3, AF.Exp)
                if c >= 4 * t:
                    # diagonal chunk: in trapezoid coords keep iff q' >= k
                    nc.gpsimd.affine_select(
                        out=ae3,
                        in_=ae3,
                        compare_op=mybir.AluOpType.is_ge,
                        fill=0.0,
                        base=0,
                        pattern=[[0, 2], [1, QTW - qlo]],
                        channel_multiplier=-1,
                    )
                if c + 2 < cmax:
                    sts[c + 2] = emit_st(c + 2)
                if plan is not None and c < len(plan):
                    for _ in range(plan[c]):
                        pump()
                for i, h in enumerate((hA, hB)):
                    nc.tensor.matmul(
                        avs[h][0 : DH + 1, qlo:QTW],
                        v_sb[:, c, h, :],
                        ae[:, i * QTW + qlo : (i + 1) * QTW],
                        start=(c == 0),
                        stop=(c == cmax - 1),
                    )
            # po=1 head first: its norm ends in a DMA; the po=0 chain ends in
            # a direct DVE multiply, keeping the block tail short.
            last = t == NQT - 1 and dc == 1
            norm_dispatch(hB, t, avs[hB], last)
            norm_dispatch(hA, t, avs[hA], last)

        def norm_dispatch(h, t, av, last=False):
            # av rows 0-63 = unnormalized out, row 64 = denominator.
            # Evacuate the PSUM accumulator FAST (den row + unnormalized out),
            # so the av bank frees after ~1.4us and the next block's A@V can
            # start; the reciprocal chain (DMA reshape [1,512]<->[128,4] so
            # the iterative-divide runs on 128 lanes) then normalizes in SBUF
            # off the critical path. For the LAST block there is no next
            # consumer of the av banks, so skip the staging copy (one less
            # DVE hop on the kernel's tail) and multiply out of PSUM; its
            # small DMAs ride the scalar queue (idle by then).
            dc, po = divmod(h, 2)
            dmae = nc.scalar if last else nc.sync
            tq = slice(t * QTW, (t + 1) * QTW)
            den = normp.tile([1, QTW], F32, name="den", tag="den")
            nc.vector.tensor_copy(den[:, :], av[DH : DH + 1, :])
            if last:
                dst = None
            elif po == 0:
                dst = outt_sb[0:DH, dc, tq]
                nc.vector.tensor_copy(dst, av[0:DH, :])
            else:
                odd = normp.tile([DH, QTW], BF16, name="odd", tag="odd")
                dst = odd[:, :]
                nc.vector.tensor_copy(dst, av[0:DH, :])
            denP = normp.tile([P, 4], F32, name="denP", tag="denP")
            dmae.dma_start(denP[:, :], den[:, :])
            recP = normp.tile([P, 4], F32, name="recP", tag="recP")
            nc.vector.reciprocal(recP[:, :], denP[:, :])
            rec = normp.tile([1, QTW], F32, name="rec", tag="rec")
            dmae.dma_start(rec[:, :], recP[:, :])
            bc = normp.tile([DH, QTW], F32, name="bc", tag="bc")
            nc.gpsimd.partition_broadcast(bc[:, :], rec[:, :], channels=DH)
            if last:
                if po == 0:
                    nc.vector.tensor_mul(outt_sb[0:DH, dc, tq], av[0:DH, :], bc[:, :])
                else:
                    odd = normp.tile([DH, QTW], BF16, name="odd", tag="odd")
                    nc.vector.tensor_mul(odd[:, :], av[0:DH, :], bc[:, :])
                    dmae.dma_start(outt_sb[DH:P, dc, tq], odd[:, :])
            else:
                nc.vector.tensor_mul(dst, dst, bc[:, :])
                if po == 1:
                    nc.sync.dma_start(outt_sb[DH:P, dc, tq], dst)

        # yT[mc, q-tile st4] = sum_dc WoT_chunk.T @ outT_chunk; DVE
        # evacuates PSUM->SBUF as bf16 (halves output DMA bytes; the host
        # gather sums the 4 partials in fp32), then DMA to DRAM. One mc
        # column at a time so the work pumps between attention chunks.
        def wo_mc(st4, mc):
            py = psmisc.tile([P, QTW], F32, name="py", tag="misc")
            for dcw in range(2):
                nc.tensor.matmul(
                    py[:, :],
                    wot_sb[:, dcw, mc * P : (mc + 1) * P],
                    outt_sb[:, dcw, st4 * QTW : (st4 + 1) * QTW],
                    start=(dcw == 0),
                    stop=(dcw == 1),
                )
            sg = sgp.tile([P, QTW], BF16, name="sg", tag="sg")
            nc.vector.tensor_copy(sg[:, :], py[:, :])
            nc.sync.dma_start(
                yt_d[mc * P : (mc + 1) * P, st4 * QTW : (st4 + 1) * QTW],
                sg[:, :],
            )

        # Split form for the LAST tile: the dcw=0 half contraction (heads of
        # dc block 0) pumps into the final attention block; only 8 matmuls +
        # adds + DMAs remain after the final norm.
        wo3_halves = {}

        def wo_mc_half0(st4, mc):
            py = psmisc.tile([P, QTW], F32, name="py", tag="misc")
            nc.tensor.matmul(
                py[:, :],
                wot_sb[:, 0, mc * P : (mc + 1) * P],
                outt_sb[:, 0, st4 * QTW : (st4 + 1) * QTW],
                start=True,
                stop=True,
            )
            sg0 = sgp.tile([P, QTW], F32, name="sg0", tag=f"sg0_{mc}", bufs=1)
            nc.vector.tensor_copy(sg0[:, :], py[:, :])
            wo3_halves[mc] = sg0

        def wo_mc_half1(st4, mc):
            py = psmisc.tile([P, QTW], F32, name="py", tag="misc")
            nc.tensor.matmul(
                py[:, :],
                wot_sb[:, 1, mc * P : (mc + 1) * P],
                outt_sb[:, 1, st4 * QTW : (st4 + 1) * QTW],
                start=True,
                stop=True,
            )
            sg = sgp.tile([P, QTW], BF16, name="sg", tag="sg")
            nc.vector.tensor_add(sg[:, :], py[:, :], wo3_halves[mc][:, :])
            nc.sync.dma_start(
                yt_d[mc * P : (mc + 1) * P, st4 * QTW : (st4 + 1) * QTW],
                sg[:, :],
            )

        # ---- main schedule ----
        # t=0 attention interleaved with the remaining projections so the
        # exp stream (ACT is the long pole) starts as early as possible;
        # attn_block(t, dc) needs only the db=dc Q/K block and V chunks
        # 0..4t+3. wo_tile(t) is emitted one block late so the PE has pair
        # work in flight while tile t's norm chains drain.
        # ---- main schedule ----
        # Just enough projection work up front for the first attention block
        # (Q/K db0 q-tile 0, V chunks 0-1), then a background queue of the
        # remaining projection groups is pumped ONE item per attention chunk,
        # so the PE queue interleaves them into the ACT-paced exp gaps
        # instead of serializing whole blocks. dc=0 blocks (db0-only) run
        # first per tile; W_o tiles one block late; last tile's W_o split
        # around the final attention block.
        def Q(db, tq4):
            b = bqs_sb[:, db : db + 1]
            return lambda: qk_group(wqt_sb, qt_sb, db, tq4, b, 0.125)

        def K(db, tq4):
            b = bk_sb[:, db : db + 1]
            return lambda: qk_group(wkt_sb, kt_sb, db, tq4, b, 1.0)

        def V(sc):
            return lambda: v_group(sc)

        def WO(t, mc):
            return lambda: wo_mc(t, mc)

        bgq = [
            V(2), V(3), K(0, 1), Q(0, 1),
            V(4), V(5), V(6), V(7), K(1, 0), Q(1, 0),
            K(0, 2), Q(0, 2), V(8), V(9),
            K(1, 1), Q(1, 1), K(0, 3), Q(0, 3),
            K(1, 2), Q(1, 2), K(1, 3), Q(1, 3), V(10), V(11),
            *(WO(0, mc) for mc in range(8)),
            *(WO(1, mc) for mc in range(8)),
            V(12), V(13),
            V(14), V(15),
            *(WO(2, mc) for mc in range(8)),
            *(lambda mc=mc: wo_mc_half0(3, mc) for mc in range(8)),
        ]

        def pump():
            if bgq:
                bgq.pop(0)()

        Q(0, 0)()
        K(0, 0)()
        v_group(0)
        v_group(1)
        # per-chunk pump plans: no pumping on a block's last two chunks (so
        # the next block's STs aren't delayed at the boundary) and none on
        # the first chunks of blocks whose pumped items depend on the
        # previous block's norms (the W_o / half0 stages).
        attn_block(0, 0, plan=[2, 2, 0, 0])
        attn_block(1, 0, plan=[2, 2, 1, 1, 1, 1, 0, 0])
        attn_block(0, 1, plan=[1, 1, 0, 0])                    # V8-9
        attn_block(2, 0, plan=[2, 2, 1, 1, 1, 1, 1, 1, 0, 0, 0, 0])  # K/Q rest + V10-11
        attn_block(1, 1, plan=[2, 2, 1, 1, 1, 1, 0, 0])        # wo tile 0
        attn_block(2, 1, plan=[0, 0, 2, 2, 1, 1, 1, 1, 1, 1, 0, 0])  # wo1 + V12-13
        attn_block(3, 0, plan=[1, 1, 1, 1, 1, 1, 1, 1, 1, 1, 0, 0, 0, 0, 0, 0])  # V14-15 + wo2
        attn_block(3, 1, plan=[0, 0, 1, 1, 1, 1, 1, 1, 1, 1, 0, 0, 0, 0, 0, 0])  # wo3 half0
        for mc in range(8):
            wo_mc_half1(3, mc)
        assert not bgq, f"{len(bgq)} background items left unpumped"


_NC_CACHE = None


def build_nc():
    global _NC_CACHE
    if _NC_CACHE is not None:
        return _NC_CACHE
    nc = bacc.Bacc("TRN2")
    xt = nc.dram_tensor("xt", [D, S], BF16, kind="ExternalInput")
    wqt = nc.dram_tensor("wqt", [D, DG], BF16, kind="ExternalInput")
    wkt = nc.dram_tensor("wkt", [D, DG], BF16, kind="ExternalInput")
    wvt = nc.dram_tensor("wvt", [D, DG], BF16, kind="ExternalInput")
    wot = nc.dram_tensor("wot", [DG, D], BF16, kind="ExternalInput")
    bq = nc.dram_tensor("bq", [DG, 1], F32, kind="ExternalInput")
    bk = nc.dram_tensor("bk", [DG, 1], F32, kind="ExternalInput")
    bv = nc.dram_tensor("bv", [1, DG], F32, kind="ExternalInput")
    yt = nc.dram_tensor("yt", [D, S], BF16, kind="ExternalOutput")
    aps = tuple(h.ap() for h in (xt, wqt, wkt, wvt, wot, bq, bk, bv, yt))
    with tile.TileContext(nc) as tc:
        _emit(tc, aps)
    nc.finalize()
    _NC_CACHE = nc
    return nc


def make_in_maps(x, W_q, b_q, W_k, b_k, W_v, b_v, W_o):
    bf = ml_dtypes.bfloat16
    in_maps = []
    for core in range(8):
        b, hg = divmod(core, 4)
        sl = slice(hg * DG, (hg + 1) * DG)
        in_maps.append(
            {
                "xt": np.ascontiguousarray(np.asarray(x)[b].T.astype(bf)),
                "wqt": np.ascontiguousarray(np.asarray(W_q)[sl, :].T.astype(bf)),
                "wkt": np.ascontiguousarray(np.asarray(W_k)[sl, :].T.astype(bf)),
                "wvt": np.ascontiguousarray(np.asarray(W_v)[sl, :].T.astype(bf)),
                "wot": np.ascontiguousarray(np.asarray(W_o)[:, sl].T.astype(bf)),
                "bq": np.ascontiguousarray(
                    np.asarray(b_q)[sl].reshape(DG, 1), dtype=np.float32
                ),
                "bk": np.ascontiguousarray(
                    np.asarray(b_k)[sl].reshape(DG, 1), dtype=np.float32
                ),
                "bv": np.ascontiguousarray(
                    np.asarray(b_v)[sl].reshape(1, DG), dtype=np.float32
                ),
            }
        )
    return in_maps


def kernel(x, W_q, b_q, W_k, b_k, W_v, b_v, W_o, b_o, _trace=False):
    global LAST_EXEC_NS, LAST_RESULTS
    nc = build_nc()
    in_maps = make_in_maps(x, W_q, b_q, W_k, b_k, W_v, b_v, W_o)
    kw = {"trace": True} if _trace else {}
    res = run_bass_kernel_spmd(nc, in_maps, core_ids=list(range(8)), **kw)
    LAST_EXEC_NS = res.exec_time_ns
    LAST_RESULTS = res
    b_o = np.asarray(b_o, dtype=np.float32)
    out = np.empty((2, S, D), np.float32)
    for b in range(2):
        ysum = sum(
            np.asarray(res.results[4 * b + g]["yt"], dtype=np.float32)
            for g in range(4)
        )
        out[b] = ysum.T + b_o
    return out


# revision 43
# speedup vs baseline: 1.2058x; 1.0151x over previous
"""Causal self-attention Trainium2 kernel (8 NeuronCores).

Sharding: core = (batch b in {0,1}, head-group hg in {0..3}); each core owns
4 of the 16 heads (256 of the 1024 q/k/v dims) for one batch element.
Data parallel over batch, tensor parallel over heads; W_o is row-parallel so
each core emits a partial output that the host sums (+ b_o) at gather time.

Device dataflow (per core), everything in "transposed" layout so the
contraction dim always sits on SBUF partitions:
  xT [1024,2048] bf16, weights pre-transposed+cast on host.
  QT/KT [d=256, s=2048] bf16 (d on partitions, 2 blocks of 128)
  V natural [s, d] bf16 with a ones-column appended per head so the A@V
  matmul also produces the softmax denominator (row 64 of the PSUM tile).

Key perf structure vs the naive version:
  - Attention processes a HEAD PAIR (po=0/po=1 of one dc block) per q tile,
    one k chunk at a time: the two K=64 ST matmuls sit at partition bases
    0/64, land on disjoint PE row groups and run CONCURRENTLY, writing one
    2-bank PSUM pair tile [128,1024]; ONE ACT Exp instruction covers both
    heads (amortizes the ~352-cycle ACTIVATE fixed cost — the exp stream is
    the attention-phase pacer, and the low PE work per exp keeps the
    kernel insensitive to the HAM PE-clock state).
  - Causal masking happens AFTER the exp (exp can't overflow: scores are
    bounded after the 1/8 scale): diagonal chunks are computed only on
    their valid trapezoid [128j:512] q-slice, and a single gpsimd
    affine_select zeroes the remaining upper triangle of both heads.
  - All projection work (Q/K groups per q-tile, V per s-chunk, W_o per
    128-row column, the split last W_o tile) is pumped through a background
    queue, one small group per attention chunk, so the strict-FIFO PE queue
    interleaves it into the ACT-paced gaps; per-block pump plans avoid the
    block boundaries and the norm-dependent first chunks.
  - Normalization: evacuate the av accumulator fast (den row + unnormalized
    out, ~1.4us) so the next block's A@V can start, then a DMA-reshaped
    [128,4] DVE reciprocal + gpsimd partition_broadcast + DVE multiply
    normalize in SBUF off the critical path.
  - QK bias+scale on DVE tensor_scalar (ACT reserved for exps); W_o
    evacuation as bf16 (host sums partials in fp32); xt streamed s-half
    first across two DMA queues so the first half of the schedule is gated
    by 2MB.
"""

import sys

for _p in ("/opt/trn_rl_repo",):
    if _p not in sys.path:
        sys.path.insert(0, _p)

import numpy as np
import ml_dtypes

import concourse.bass as bass
import concourse.bacc as bacc
import concourse.mybir as mybir
from concourse import tile
from concourse.bass_utils import run_bass_kernel_spmd

P = 128
S = 2048  # sequence length
D = 1024  # d_model
DG = 256  # dims per head-group (4 heads x 64)
DH = 64   # head dim
NHG = 4   # heads per core
EC = D // P   # 8 contraction chunks over d_model
KC = S // P   # 16 key chunks
QTW = 512     # q tile width
NQT = S // QTW  # 4 q tiles
W2 = 2 * QTW  # pair-tile width (2 PSUM banks)
F32 = mybir.dt.float32
BF16 = mybir.dt.bfloat16
AF = mybir.ActivationFunctionType

LAST_EXEC_NS = None
LAST_RESULTS = None


def _emit(tc, aps):
    nc = tc.nc
    xt_d, wqt_d, wkt_d, wvt_d, wot_d, bq_d, bk_d, bv_d, yt_d = aps

    with (
        tc.tile_pool(name="const", bufs=1) as constp,
        tc.tile_pool(name="wpool", bufs=1) as wp,
        tc.tile_pool(name="xpool", bufs=1) as xp,
        tc.tile_pool(name="qkvp", bufs=1) as qkvp,
        tc.tile_pool(name="aep", bufs=4) as aep,
        tc.tile_pool(name="outp", bufs=1) as outp,
        tc.tile_pool(name="normp", bufs=2) as normp,
        tc.tile_pool(name="sgp", bufs=3) as sgp,
        tc.tile_pool(name="pspair", bufs=2, space="PSUM") as pspair,
        tc.tile_pool(name="psav", bufs=2, space="PSUM") as psav,
        tc.tile_pool(name="psmisc", bufs=2, space="PSUM") as psmisc,
    ):
        # ---- persistent SBUF tensors ----
        bq_sb = constp.tile([P, 2], F32, name="bq_sb")
        bqs_sb = constp.tile([P, 2], F32, name="bqs_sb")
        bk_sb = constp.tile([P, 2], F32, name="bk_sb")
        bv1_sb = constp.tile([1, DG], F32, name="bv1_sb")
        bvb_sb = constp.tile([P, DG], F32, name="bvb_sb")

        wqt_sb = wp.tile([P, EC, DG], BF16, name="wqt_sb")
        wkt_sb = wp.tile([P, EC, DG], BF16, name="wkt_sb")
        wvt_sb = wp.tile([P, EC, DG], BF16, name="wvt_sb")
        wot_sb = wp.tile([P, 2, D], BF16, name="wot_sb")

        xt_sb = xp.tile([P, EC, S], BF16, name="xt_sb")

        qt_sb = qkvp.tile([P, 2, S], BF16, name="qt_sb")
        kt_sb = qkvp.tile([P, 2, S], BF16, name="kt_sb")
        v_sb = qkvp.tile([P, KC, NHG, DH + 1], BF16, name="v_sb")

        outt_sb = outp.tile([P, 2, S], BF16, name="outt_sb")

        # ---- input DMAs ----
        # xt goes FIRST on the sync queue (in s-halves for finer-grained
        # streaming into the Q/K ec-loops); constants and all weight chunks
        # issue in parallel from the gpsimd queue, so the xt stream is gated
        # by nothing.
        # xt in four BATCHED s-quarter transfers (1MB each; per-chunk DMAs
        # fragment across the DGE round-robin and the last chunks land ~10us
        # late). Quarter 0 alone gates the whole first attention block
        # (Q00/K00/V0-3 all read s-cols 0-511); quarter 1 (q tile 1 /
        # k chunks 4-7) streams in parallel on the scalar queue.
        for qi in range(4):
            eng = nc.sync if qi % 2 == 0 else nc.scalar
            eng.dma_start(
                xt_sb[:, :, qi * QTW : (qi + 1) * QTW],
                xt_d[:, qi * QTW : (qi + 1) * QTW]
                .rearrange("(ec p) s -> p ec s", p=P),
            )
        for c in range(2):
            nc.gpsimd.dma_start(bq_sb[:, c : c + 1], bq_d[c * P : (c + 1) * P, :])
            nc.gpsimd.dma_start(bk_sb[:, c : c + 1], bk_d[c * P : (c + 1) * P, :])
        nc.gpsimd.dma_start(bv1_sb[:, :], bv_d[:, :])
        nc.scalar.mul(bqs_sb[:, :], bq_sb[:, :], 0.125)
        nc.gpsimd.partition_broadcast(bvb_sb[:, :], bv1_sb[:, :], channels=P)
        # ones column in V for the fused softmax denominator
        nc.vector.memset(v_sb[:, :, :, DH : DH + 1], 1.0)
        # batched weight DMAs, db=0 halves of Q/K first (the only weights
        # the first attention block needs — keeps the critical input set to
        # xt quarter 0 + 0.5MB of weights)
        for db in range(2):
            nc.gpsimd.dma_start(
                wqt_sb[:, :, db * P : (db + 1) * P],
                wqt_d[:, db * P : (db + 1) * P]
                .rearrange("(ec p) d -> p ec d", p=P),
            )
            nc.gpsimd.dma_start(
                wkt_sb[:, :, db * P : (db + 1) * P],
                wkt_d[:, db * P : (db + 1) * P]
                .rearrange("(ec p) d -> p ec d", p=P),
            )
        nc.gpsimd.dma_start(
            wvt_sb[:, :, :], wvt_d[:, :].rearrange("(ec p) d -> p ec d", p=P)
        )
        nc.gpsimd.dma_start(
            wot_sb[:, :, :], wot_d[:, :].rearrange("(c p) m -> p c m", p=P)
        )

        # ---- QKV projections (single-bank groups on the shared "misc"
        # ring, so they can be pumped one at a time between attention
        # chunks and the PE queue interleaves them into the ACT-paced
        # gaps) ----
        def qk_group(w_sb, dst_sb, db, tq4, bias_ap, scale):
            ps = psmisc.tile([P, QTW], F32, name="pqk", tag="misc")
            for ec in range(EC):
                nc.tensor.matmul(
                    ps[:, :],
                    w_sb[:, ec, db * P : (db + 1) * P],
                    xt_sb[:, ec, tq4 * QTW : (tq4 + 1) * QTW],
                    start=(ec == 0),
                    stop=(ec == EC - 1),
                )
            # bias+scale on DVE (tensor_scalar with per-partition scalar AP)
            # so ACT stays reserved for the exp stream.
            nc.vector.tensor_scalar(
                dst_sb[:, db, tq4 * QTW : (tq4 + 1) * QTW],
                ps[:, :],
                scale,
                bias_ap,
                mybir.AluOpType.mult,
                mybir.AluOpType.add,
            )

        # V natural for one s-chunk; DVE adds the (partition-broadcast)
        # bias while casting to bf16 into v_sb.
        def v_group(sc):
            pv = psmisc.tile([P, QTW], F32, name="pv", tag="misc")
            for ec in range(EC):
                nc.tensor.matmul(
                    pv[:, 0:DG],
                    xt_sb[:, ec, sc * P : (sc + 1) * P],
                    wvt_sb[:, ec, :],
                    start=(ec == 0),
                    stop=(ec == EC - 1),
                )
            nc.vector.tensor_add(
                v_sb[:, sc, :, 0:DH],
                pv[:, 0:DG].rearrange("p (h d) -> p h d", h=NHG),
                bvb_sb[:, :].rearrange("p (h d) -> p h d", h=NHG),
            )

        # ---- attention: a head PAIR (po=0, po=1 of one dc block) on one q
        # tile, one k-chunk at a time. The two ST matmuls have K=64 and
        # partition bases 0 / 64, so they land on disjoint PE row groups
        # (tile_position (0,0) / (64,0)) and run CONCURRENTLY; both write one
        # [128,1024] PSUM pair tile -> one Exp ACTIVATE covers both heads ->
        # (diagonal chunks) one gpsimd affine_select zeroes the future
        # entries of both halves -> two A@V matmuls into the two per-head
        # accumulators. PE work per exp is ~3 matmul slots, low enough that
        # even at the cold (K=4/8) PE clock the ACT exp stream stays the
        # pacer, so HAM state stops mattering in this phase.
        def attn_block(t, dc, plan=None):
            hA, hB = 2 * dc, 2 * dc + 1
            cmax = 4 * t + 4
            avs = {
                h: psav.tile([P, QTW], F32, name=f"av{h}", tag="av")
                for h in (hA, hB)
            }

            # Diagonal chunks (c = 4t+j) only need q >= 128j: the ST/exp/
            # affine/AV all run on the trapezoid [qlo:512] slice, qlo = 128j.
            def qlo_of(c):
                return 128 * (c - 4 * t) if c >= 4 * t else 0

            def emit_st(c):
                stp = pspair.tile([P, W2], F32, name="stp", tag="pp")
                qlo = qlo_of(c)
                for i, h in enumerate((hA, hB)):
                    qoff = (h % 2) * DH
                    nc.tensor.matmul(
                        stp[:, i * QTW + qlo : (i + 1) * QTW],
                        kt_sb[qoff : qoff + DH, dc, c * P : (c + 1) * P],
                        qt_sb[qoff : qoff + DH, dc, t * QTW + qlo : (t + 1) * QTW],
                        start=True,
                        stop=True,
                    )
                return stp

            sts = {0: emit_st(0)}
            if cmax > 1:
                sts[1] = emit_st(1)
            for c in range(cmax):
                qlo = qlo_of(c)
                ae = aep.tile([P, W2], BF16, name="ae", tag="ae")
                ae3 = ae[:, :].rearrange("k (h q) -> k h q", h=2)[:, :, qlo:QTW]
                st3 = sts[c][:, :].rearrange("k (h q) -> k h q", h=2)[:, :, qlo:QTW]
                nc.scalar.activation(ae3, st3, AF.Exp)
                if c >= 4 * t:
                    # diagonal chunk: in trapezoid coords keep iff q' >= k
                    nc.gpsimd.affine_select(
                        out=ae3,
                        in_=ae3,
                        compare_op=mybir.AluOpType.is_ge,
                        fill=0.0,
                        base=0,
                        pattern=[[0, 2], [1, QTW - qlo]],
                        channel_multiplier=-1,
                    )
                if c + 2 < cmax:
                    sts[c + 2] = emit_st(c + 2)
                if plan is not None and c < len(plan):
                    for _ in range(plan[c]):
                        pump()
                for i, h in enumerate((hA, hB)):
                    nc.tensor.matmul(
                        avs[h][0 : DH + 1, qlo:QTW],
                        v_sb[:, c, h, :],
                        ae[:, i * QTW + qlo : (i + 1) * QTW],
                        start=(c == 0),
                        stop=(c == cmax - 1),
                    )
            # po=1 head first: its norm ends in a DMA; the po=0 chain ends in
            # a direct DVE multiply, keeping the block tail short.
            last = t == NQT - 1 and dc == 1
            norm_dispatch(hB, t, avs[hB], last)
            norm_dispatch(hA, t, avs[hA], last)

        def norm_dispatch(h, t, av, last=False):
            # av rows 0-63 = unnormalized out, row 64 = denominator.
            # Evacuate the PSUM accumulator FAST (den row + unnormalized out),
            # so the av bank frees after ~1.4us and the next block's A@V can
            # start; the reciprocal chain (DMA reshape [1,512]<->[128,4] so
            # the iterative-divide runs on 128 lanes) then normalizes in SBUF
            # off the critical path. For the LAST block there is no next
            # consumer of the av banks, so skip the staging copy (one less
            # DVE hop on the kernel's tail) and multiply out of PSUM; its
            # small DMAs ride the scalar queue (idle by then).
            dc, po = divmod(h, 2)
            dmae = nc.scalar if last else nc.sync
            tq = slice(t * QTW, (t + 1) * QTW)
            den = normp.tile([1, QTW], F32, name="den", tag="den")
            nc.vector.tensor_copy(den[:, :], av[DH : DH + 1, :])
            if last:
                dst = None
            elif po == 0:
                dst = outt_sb[0:DH, dc, tq]
                nc.vector.tensor_copy(dst, av[0:DH, :])
            else:
                odd = normp.tile([DH, QTW], BF16, name="odd", tag="odd")
                dst = odd[:, :]
                nc.vector.tensor_copy(dst, av[0:DH, :])
            denP = normp.tile([P, 4], F32, name="denP", tag="denP")
            dmae.dma_start(denP[:, :], den[:, :])
            recP = normp.tile([P, 4], F32, name="recP", tag="recP")
            nc.vector.reciprocal(recP[:, :], denP[:, :])
            rec = normp.tile([1, QTW], F32, name="rec", tag="rec")
            dmae.dma_start(rec[:, :], recP[:, :])
            bc = normp.tile([DH, QTW], F32, name="bc", tag="bc")
            nc.gpsimd.partition_broadcast(bc[:, :], rec[:, :], channels=DH)
            if last:
                if po == 0:
                    nc.vector.tensor_mul(outt_sb[0:DH, dc, tq], av[0:DH, :], bc[:, :])
                else:
                    odd = normp.tile([DH, QTW], BF16, name="odd", tag="odd")
                    nc.vector.tensor_mul(odd[:, :], av[0:DH, :], bc[:, :])
                    dmae.dma_start(outt_sb[DH:P, dc, tq], odd[:, :])
            else:
                nc.vector.tensor_mul(dst, dst, bc[:, :])
                if po == 1:
                    nc.sync.dma_start(outt_sb[DH:P, dc, tq], dst)

        # yT[mc, q-tile st4] = sum_dc WoT_chunk.T @ outT_chunk; DVE
        # evacuates PSUM->SBUF as bf16 (halves output DMA bytes; the host
        # gather sums the 4 partials in fp32), then DMA to DRAM. One mc
        # column at a time so the work pumps between attention chunks.
        def wo_mc(st4, mc):
            py = psmisc.tile([P, QTW], F32, name="py", tag="misc")
            for dcw in range(2):
                nc.tensor.matmul(
                    py[:, :],
                    wot_sb[:, dcw, mc * P : (mc + 1) * P],
                    outt_sb[:, dcw, st4 * QTW : (st4 + 1) * QTW],
                    start=(dcw == 0),
                    stop=(dcw == 1),
                )
            sg = sgp.tile([P, QTW], BF16, name="sg", tag="sg")
            nc.vector.tensor_copy(sg[:, :], py[:, :])
            nc.sync.dma_start(
                yt_d[mc * P : (mc + 1) * P, st4 * QTW : (st4 + 1) * QTW],
                sg[:, :],
            )

        # Split form for the LAST tile: the dcw=0 half contraction (heads of
        # dc block 0) pumps into the final attention block; only 8 matmuls +
        # adds + DMAs remain after the final norm.
        wo3_halves = {}

        def wo_mc_half0(st4, mc):
            py = psmisc.tile([P, QTW], F32, name="py", tag="misc")
            nc.tensor.matmul(
                py[:, :],
                wot_sb[:, 0, mc * P : (mc + 1) * P],
                outt_sb[:, 0, st4 * QTW : (st4 + 1) * QTW],
                start=True,
                stop=True,
            )
            sg0 = sgp.tile([P, QTW], F32, name="sg0", tag=f"sg0_{mc}", bufs=1)
            nc.vector.tensor_copy(sg0[:, :], py[:, :])
            wo3_halves[mc] = sg0

        def wo_mc_half1(st4, mc):
            py = psmisc.tile([P, QTW], F32, name="py", tag="misc")
            nc.tensor.matmul(
                py[:, :],
                wot_sb[:, 1, mc * P : (mc + 1) * P],
                outt_sb[:, 1, st4 * QTW : (st4 + 1) * QTW],
                start=True,
                stop=True,
            )
            sg = sgp.tile([P, QTW], BF16, name="sg", tag="sg")
            nc.vector.tensor_add(sg[:, :], py[:, :], wo3_halves[mc][:, :])
            nc.sync.dma_start(
                yt_d[mc * P : (mc + 1) * P, st4 * QTW : (st4 + 1) * QTW],
                sg[:, :],
            )

        # ---- main schedule ----
        # t=0 attention interleaved with the remaining projections so the
        # exp stream (ACT is the long pole) starts as early as possible;
        # attn_block(t, dc) needs only the db=dc Q/K block and V chunks
        # 0..4t+3. wo_tile(t) is emitted one block late so the PE has pair
        # work in flight while tile t's norm chains drain.
        # ---- main schedule ----
        # Just enough projection work up front for the first attention block
        # (Q/K db0 q-tile 0, V chunks 0-1), then a background queue of the
        # remaining projection groups is pumped ONE item per attention chunk,
        # so the PE queue interleaves them into the ACT-paced exp gaps
        # instead of serializing whole blocks. dc=0 blocks (db0-only) run
        # first per tile; W_o tiles one block late; last tile's W_o split
        # around the final attention block.
        def Q(db, tq4):
            b = bqs_sb[:, db : db + 1]
            return lambda: qk_group(wqt_sb, qt_sb, db, tq4, b, 0.125)

        def K(db, tq4):
            b = bk_sb[:, db : db + 1]
            return lambda: qk_group(wkt_sb, kt_sb, db, tq4, b, 1.0)

        def V(sc):
            return lambda: v_group(sc)

        def WO(t, mc):
            return lambda: wo_mc(t, mc)

        bgq = [
            V(2), V(3), K(0, 1), Q(0, 1),
            V(4), V(5), V(6), V(7), K(1, 0), Q(1, 0),
            K(0, 2), Q(0, 2), V(8), V(9),
            K(1, 1), Q(1, 1), K(0, 3), Q(0, 3),
            K(1, 2), Q(1, 2), K(1, 3), Q(1, 3), V(10), V(11),
            *(WO(0, mc) for mc in range(8)),
            *(WO(1, mc) for mc in range(8)),
            V(12), V(13),
            V(14), V(15),
            *(WO(2, mc) for mc in range(8)),
            *(lambda mc=mc: wo_mc_half0(3, mc) for mc in range(8)),
        ]

        def pump():
            if bgq:
                bgq.pop(0)()

        Q(0, 0)()
        K(0, 0)()
        v_group(0)
        v_group(1)
        # per-chunk pump plans: no pumping on a block's last two chunks (so
        # the next block's STs aren't delayed at the boundary) and none on
        # the first chunks of blocks whose pumped items depend on the
        # previous block's norms (the W_o / half0 stages).
        attn_block(0, 0, plan=[2, 2, 0, 0])
        attn_block(1, 0, plan=[2, 2, 1, 1, 1, 1, 0, 0])
        attn_block(0, 1, plan=[1, 1, 0, 0])                    # V8-9
        attn_block(2, 0, plan=[2, 2, 1, 1, 1, 1, 1, 1, 0, 0, 0, 0])  # K/Q rest + V10-11
        attn_block(1, 1, plan=[2, 2, 1, 1, 1, 1, 0, 0])        # wo tile 0
        attn_block(2, 1, plan=[0, 0, 2, 2, 1, 1, 1, 1, 1, 1, 0, 0])  # wo1 + V12-13
        attn_block(3, 0, plan=[1, 1, 1, 1, 1, 1, 1, 1, 1, 1, 0, 0, 0, 0, 0, 0])  # V14-15 + wo2
        attn_block(3, 1, plan=[0, 0, 1, 1, 1, 1, 1, 1, 1, 1, 0, 0, 0, 0, 0, 0])  # wo3 half0
        for mc in range(8):
            wo_mc_half1(3, mc)
        assert not bgq, f"{len(bgq)} background items left unpumped"


_NC_CACHE = None


def build_nc():
    global _NC_CACHE
    if _NC_CACHE is not None:
        return _NC_CACHE
    nc = bacc.Bacc("TRN2")
    xt = nc.dram_tensor("xt", [D, S], BF16, kind="ExternalInput")
    wqt = nc.dram_tensor("wqt", [D, DG], BF16, kind="ExternalInput")
    wkt = nc.dram_tensor("wkt", [D, DG], BF16, kind="ExternalInput")
    wvt = nc.dram_tensor("wvt", [D, DG], BF16, kind="ExternalInput")
    wot = nc.dram_tensor("wot", [DG, D], BF16, kind="ExternalInput")
    bq = nc.dram_tensor("bq", [DG, 1], F32, kind="ExternalInput")
    bk = nc.dram_tensor("bk", [DG, 1], F32, kind="ExternalInput")
    bv = nc.dram_tensor("bv", [1, DG], F32, kind="ExternalInput")
    yt = nc.dram_tensor("yt", [D, S], BF16, kind="ExternalOutput")
    aps = tuple(h.ap() for h in (xt, wqt, wkt, wvt, wot, bq, bk, bv, yt))
    with tile.TileContext(nc) as tc:
        _emit(tc, aps)
    nc.finalize()
    _NC_CACHE = nc
    return nc


def make_in_maps(x, W_q, b_q, W_k, b_k, W_v, b_v, W_o):
    bf = ml_dtypes.bfloat16
    in_maps = []
    for core in range(8):
        b, hg = divmod(core, 4)
        sl = slice(hg * DG, (hg + 1) * DG)
        in_maps.append(
            {
                "xt": np.ascontiguousarray(np.asarray(x)[b].T.astype(bf)),
                "wqt": np.ascontiguousarray(np.asarray(W_q)[sl, :].T.astype(bf)),
                "wkt": np.ascontiguousarray(np.asarray(W_k)[sl, :].T.astype(bf)),
                "wvt": np.ascontiguousarray(np.asarray(W_v)[sl, :].T.astype(bf)),
                "wot": np.ascontiguousarray(np.asarray(W_o)[:, sl].T.astype(bf)),
                "bq": np.ascontiguousarray(
                    np.asarray(b_q)[sl].reshape(DG, 1), dtype=np.float32
                ),
                "bk": np.ascontiguousarray(
                    np.asarray(b_k)[sl].reshape(DG, 1), dtype=np.float32
                ),
                "bv": np.ascontiguousarray(
                    np.asarray(b_v)[sl].reshape(1, DG), dtype=np.float32
                ),
            }
        )
    return in_maps


def kernel(x, W_q, b_q, W_k, b_k, W_v, b_v, W_o, b_o, _trace=False):
    global LAST_EXEC_NS, LAST_RESULTS
    nc = build_nc()
    in_maps = make_in_maps(x, W_q, b_q, W_k, b_k, W_v, b_v, W_o)
    kw = {"trace": True} if _trace else {}
    res = run_bass_kernel_spmd(nc, in_maps, core_ids=list(range(8)), **kw)
    LAST_EXEC_NS = res.exec_time_ns
    LAST_RESULTS = res
    b_o = np.asarray(b_o, dtype=np.float32)
    out = np.empty((2, S, D), np.float32)
    for b in range(2):
        ysum = sum(
            np.asarray(res.results[4 * b + g]["yt"], dtype=np.float32)
            for g in range(4)
        )
        out[b] = ysum.T + b_o
    return out
